# revision 77
# baseline (speedup 1.0000x reference)
"""Trainium2 Bass kernel for nn_Attention_43413529428606 (linear attention
with l2-normed q/k, interleaved RoPE, mask, per-head power scaling).

v2: mask compaction.  Only the ~50% unmasked rows are processed: the host
gathers rows with mask==1 per batch, splits them over the 4 cores of that
batch (cores 0-3 batch 0, 4-7 batch 1; capacity 1024 rows/core), and
scatters the output back (masked rows are zero by construction).  Each core
projects k/v for its rows, applies l2norm+RoPE, accumulates the per-head
transposed Gram state kvT = sum_c v ⊗ khat, AllReduces it in fp16 (256 KB)
within its batch group, folds Wo into the state (G = kvT^T @ WoT per head
pair), then computes q/norm/rope and the fused attn+out projection
out = qhat @ G for its rows.  Activation-engine usage is restricted to
{Copy, Sqrt} (one table set; 1/sqrt runs as Act-Sqrt + DVE-reciprocal),
squares and sums run on DVE/Pool, and the load DMAs are ordered so the
first projection starts after ~1.5 MiB.  The two AllReduce halves overlap
the second half of phase A and the whole q pipeline.

Self-contained: hardcodes all shapes; no sibling imports.
"""

import sys

for _p in ("/opt/trn_rl_repo",):
    if _p not in sys.path:
        sys.path.append(_p)

from contextlib import ExitStack

import numpy as np

import concourse.bass as bass
import concourse.bacc as bacc
import concourse.tile as tile
from concourse import mybir
from concourse.bass_utils import run_bass_kernel_spmd

F32 = mybir.dt.float32
F16 = mybir.dt.float16

DIM = 1024
H = 16
HD = 64
B = 2
C = 8192
ROPE_THETA = 10000.0

N_CORES = 8
R = 1024  # compacted rows per core (capacity; ~n_unmasked/4)
NC_T = R // 128  # 8 c-tiles of 128 (phase A)
NQ_T = R // 512  # 2 c-supertiles of 512 (phase B)
ND = DIM // 128  # 8 d-chunks
NJ = DIM // 128  # 8 j-tiles
NPAIR = H // 2  # 8 head pairs

Copy = mybir.ActivationFunctionType.Copy
Sqrt = mybir.ActivationFunctionType.Sqrt
MUL = mybir.AluOpType.mult
ADD = mybir.AluOpType.add


def build_nc(sim_mode=False, phases="ABC", reps=1, coll=True):
    nc = bacc.Bacc(
        "TRN2",
        target_bir_lowering=False,
        debug=False,
        num_devices=1 if sim_mode else N_CORES,
    )

    # ---- DRAM parameters (per-core shapes, fp16 data path) ----
    xT = nc.dram_tensor("xT", [DIM, R], F16, kind="ExternalInput").ap()
    WkT = nc.dram_tensor("WkT", [DIM, DIM], F16, kind="ExternalInput").ap()
    WvT = nc.dram_tensor("WvT", [DIM, DIM], F16, kind="ExternalInput").ap()
    WqT = nc.dram_tensor("WqT", [DIM, DIM], F16, kind="ExternalInput").ap()
    WoT = nc.dram_tensor("WoT", [DIM, DIM], F16, kind="ExternalInput").ap()
    cosC = nc.dram_tensor("cosC", [R, HD], F16, kind="ExternalInput").ap()
    sinC = nc.dram_tensor("sinC", [R, HD], F16, kind="ExternalInput").ap()
    cosF = nc.dram_tensor("cosF", [128, R], F16, kind="ExternalInput").ap()
    sinF = nc.dram_tensor("sinF", [128, R], F16, kind="ExternalInput").ap()
    maskC = nc.dram_tensor("maskC", [128, NC_T], F32, kind="ExternalInput").ap()
    ind16T = nc.dram_tensor("ind16T", [DIM, 16], F16, kind="ExternalInput").ap()
    ind16 = nc.dram_tensor("ind16", [16, DIM], F16, kind="ExternalInput").ap()
    Pmat = nc.dram_tensor("Pmat", [128, 128], F16, kind="ExternalInput").ap()

    kv_in_d = [
        nc.dram_tensor(f"kv_in_d{h}", [128, NPAIR * 128], F16) for h in range(2)
    ]
    kv_out_d = [
        nc.dram_tensor(f"kv_out_d{h}", [128, NPAIR * 128], F16) for h in range(2)
    ]

    out_d = nc.dram_tensor("out", [DIM, R], F16, kind="ExternalOutput").ap()

    def blkview(dram_ap, csl):
        return dram_ap.rearrange("(t p) c -> p t c", p=128)[:, :, csl]

    with tile.TileContext(nc) as tc:
        with ExitStack() as ctx:
            consts = ctx.enter_context(tc.tile_pool(name="consts", bufs=1))

            cosC_t = consts.tile([128, NC_T * HD], F16, tag="cosC")
            sinC_t = consts.tile([128, NC_T * HD], F16, tag="sinC")
            maskC_t = consts.tile([128, NC_T], F32, tag="maskC")
            ind16T_t = consts.tile([128, NJ * 16], F16, tag="ind16T")
            ind16_t = consts.tile([16, DIM], F16, tag="ind16")
            P_t = consts.tile([128, 128], F16, tag="Pmat")
            consts_emitted = [False]

            def _emit_consts():
                if consts_emitted[0]:
                    return
                consts_emitted[0] = True
                nc.sync.dma_start(
                    out=cosC_t[:].rearrange("p (t f) -> p t f", t=NC_T),
                    in_=cosC[:].rearrange("(t p) f -> p t f", p=128),
                )
                nc.sync.dma_start(
                    out=sinC_t[:].rearrange("p (t f) -> p t f", t=NC_T),
                    in_=sinC[:].rearrange("(t p) f -> p t f", p=128),
                )
                nc.sync.dma_start(out=maskC_t[:], in_=maskC[:])
                nc.sync.dma_start(
                    out=ind16T_t[:].rearrange("p (t f) -> p t f", t=NJ),
                    in_=ind16T[:].rearrange("(t p) f -> p t f", p=128),
                )
                nc.sync.dma_start(out=ind16_t[:], in_=ind16[:])
                nc.sync.dma_start(out=P_t[:], in_=Pmat[:])

            for _rep in range(reps):
              with ExitStack() as ctxX:
                xpool = ctxX.enter_context(tc.tile_pool(name="xpool", bufs=1))
                xT_all = xpool.tile([128, ND * R], F16, tag="xT")

                def _xdma(xc, eng=None):
                    csl = slice(xc * 256, (xc + 1) * 256)
                    (eng or nc.sync).dma_start(
                        out=xT_all[:]
                        .rearrange("p (t c) -> p t c", t=ND)[:, :, csl],
                        in_=xT[:, csl].rearrange("(t p) c -> p t c", p=128),
                    )

                def _wdma(wt, wsrc, jc, eng=None):
                    jsl = slice(jc * 512, (jc + 1) * 512)
                    (eng or nc.sync).dma_start(
                        out=wt[:].rearrange("p (t f) -> p t f", t=ND)[:, :, jsl],
                        in_=wsrc[:].rearrange("(t p) f -> p t f", p=128)[
                            :, :, jsl
                        ],
                    )

                def xsl(dc, csl):
                    lo = dc * R
                    return xT_all[:, lo + csl.start : lo + csl.stop]

                if "B" in phases and "C" in phases:
                    wBC = ctxX.enter_context(tc.tile_pool(name="wBC", bufs=1))
                    wq_all = wBC.tile([128, ND * DIM], F16, tag="wq")
                    wo_all = wBC.tile([128, ND * DIM], F16, tag="wo")
                    cosF_t = wBC.tile([128, R], F16, tag="cosF")
                    sinF_t = wBC.tile([128, R], F16, tag="sinF")
                    G_pool = ctxX.enter_context(
                        tc.tile_pool(name="G_pool", bufs=1)
                    )
                    G_all = G_pool.tile([128, NJ * DIM], F16, tag="G")

                def _emit_phaseB_loads():
                    if not ("B" in phases and "C" in phases):
                        return
                    nc.sync.dma_start(
                        out=wq_all[:].rearrange("p (t f) -> p t f", t=ND),
                        in_=WqT[:].rearrange("(t p) f -> p t f", p=128),
                    )
                    nc.sync.dma_start(
                        out=wo_all[:].rearrange("p (t f) -> p t f", t=ND),
                        in_=WoT[:].rearrange("(t p) f -> p t f", p=128),
                    )
                    nc.sync.dma_start(out=cosF_t[:], in_=cosF[:])
                    nc.sync.dma_start(out=sinF_t[:], in_=sinF[:])

                # pools shared between late phase A and pass1 of phase B:
                # psB gets PSUM banks 0-1, ahead of psA, so the first q
                # projections can overlap the phase-A Gram tail
                haveB = "B" in phases and "C" in phases
                ctxO = ctxX.enter_context(ExitStack())
                if haveB:
                    psB = ctxO.enter_context(
                        tc.tile_pool(name="psB", bufs=2, space="PSUM")
                    )
                    qSP = ctxO.enter_context(tc.tile_pool(name="qSP", bufs=2))
                    sqP = ctxO.enter_context(
                        tc.tile_pool(name="sqP", bufs=NJ + 1)
                    )
                    # current supertile's contiguous q buffer (pass1 evicts
                    # into slices; one DMA pair-swaps the whole thing)
                    qall_cur = [None]

                    def _emit_pass1_jt(ct_, jt):
                        """q proj + eviction + square for one j-tile."""
                        cs = slice(ct_ * 512, (ct_ + 1) * 512)
                        jlo = jt * 128
                        q_ps = psB.tile([128, 512], F32, tag="q_ps")
                        for dc in range(ND):
                            nc.tensor.matmul(
                                q_ps[:],
                                wq_all[
                                    :, dc * DIM + jlo : dc * DIM + jlo + 128
                                ],
                                xsl(dc, cs),
                                start=(dc == 0),
                                stop=(dc == ND - 1),
                            )
                        q_sb = qall_cur[0][:, jt * 512 : (jt + 1) * 512]
                        nc.scalar.activation(q_sb, q_ps[:], Copy)
                        sq = sqP.tile([128, 512], F16, tag="sqB")
                        nc.vector.tensor_mul(sq[:], q_sb, q_sb)
                        return q_sb, sq

                # ========= Phase A: k/v proj + process + kvT Grams ==========
                with ExitStack() as ctxA:
                  if "A" not in phases:
                    _emit_consts()
                    for xc in range(4):
                        _xdma(xc)
                    _emit_phaseB_loads()
                    p1_early = []

                    def _emit_kv_tail():
                        pass
                  else:
                    wA = ctxA.enter_context(tc.tile_pool(name="wA", bufs=1))
                    psA = ctxA.enter_context(
                        tc.tile_pool(name="psA", bufs=2, space="PSUM")
                    )
                    pskv = ctxA.enter_context(
                        tc.tile_pool(name="pskv", bufs=1, space="PSUM")
                    )
                    sbA = ctxA.enter_context(tc.tile_pool(name="sbA", bufs=2))
                    sb1 = ctxA.enter_context(tc.tile_pool(name="sb1", bufs=2))
                    smA = ctxA.enter_context(tc.tile_pool(name="smA", bufs=2))

                    # explicit load order on the SP dma queue: everything the
                    # phase-A pipeline needs, j/c-chunked so compute starts
                    # after ~1.5 MiB, then the phase-B weights
                    wk_all = wA.tile([128, ND * DIM], F16, tag="wk")
                    wv_all = wA.tile([128, ND * DIM], F16, tag="wv")
                    # first load on the idle Act queue (shorter preamble
                    # than SP), rest on SP in priority order
                    _wdma(wk_all, WkT, 0, eng=nc.scalar)
                    _xdma(0)
                    _wdma(wv_all, WvT, 0)
                    _wdma(wk_all, WkT, 1)
                    _xdma(1)
                    _emit_consts()
                    _wdma(wv_all, WvT, 1)
                    _xdma(2)
                    _xdma(3)
                    _emit_phaseB_loads()

                    kv_ps = pskv.tile([128, NPAIR * 128], F32, tag="kvps")
                    kv_pending = []
                    HALF_T = NC_T // 2

                    # kvT Gram: out[e,d] = sum_c v[c,e] khat[c,d].
                    # Accumulated in two c-halves; each half is AllReduced
                    # separately so the first collective overlaps the second
                    # half of phase A.  On HW start=True zeroes the whole
                    # PSUM bank, so only the first pair written to each bank
                    # per half may carry it.
                    def _emit_kv(item):
                        ct_, khat_, v_ = item
                        first = ct_ % HALF_T == 0
                        last = ct_ % HALF_T == HALF_T - 1
                        for p in range(NPAIR):
                            ps_ = slice(p * 128, (p + 1) * 128)
                            nc.tensor.matmul(
                                kv_ps[:, ps_],
                                v_[:, ps_],
                                khat_[:, ps_],
                                start=(
                                    True if sim_mode else (first and p % 4 == 0)
                                ),
                                stop=(True if sim_mode else last),
                            )
                        if last:
                            _emit_kv_flush(ct_ // HALF_T)

                    def _emit_kv_flush(h):
                        kv_sb = sbA.tile([128, NPAIR * 128], F16, tag="kv_sb")
                        nc.vector.tensor_copy(kv_sb[:], kv_ps[:])
                        nc.sync.dma_start(out=kv_in_d[h].ap(), in_=kv_sb[:])
                        if sim_mode or not coll:
                            nc.sync.dma_start(
                                out=kv_out_d[h].ap(), in_=kv_in_d[h].ap()
                            )
                        else:
                            nc.gpsimd.collective_compute(
                                "AllReduce",
                                ADD,
                                replica_groups=[[0, 1, 2, 3], [4, 5, 6, 7]],
                                ins=[kv_in_d[h].ap().opt()],
                                outs=[kv_out_d[h].ap().opt()],
                            )

                    for ct in range(NC_T):
                        cs = slice(ct * 128, (ct + 1) * 128)
                        k_ps = psA.tile([128, DIM], F32, tag="proj_ps")
                        v_ps = psA.tile([128, DIM], F32, tag="proj_ps")
                        for half in range(2):
                            js = slice(half * 512, (half + 1) * 512)
                            for dc in range(ND):
                                nc.tensor.matmul(
                                    k_ps[:, js],
                                    xsl(dc, cs),
                                    wk_all[
                                        :, dc * DIM + js.start : dc * DIM + js.stop
                                    ],
                                    start=(dc == 0),
                                    stop=(dc == ND - 1),
                                )
                            for dc in range(ND):
                                nc.tensor.matmul(
                                    v_ps[:, js],
                                    xsl(dc, cs),
                                    wv_all[
                                        :, dc * DIM + js.start : dc * DIM + js.stop
                                    ],
                                    start=(dc == 0),
                                    stop=(dc == ND - 1),
                                )

                        # evictions: v and k on Act (cast fp16)
                        v_sb = sbA.tile([128, DIM], F16, tag="v_sb")
                        nc.scalar.activation(v_sb[:], v_ps[:], Copy)
                        k_sb = sbA.tile([128, DIM], F16, tag="k_sb")
                        nc.scalar.activation(k_sb[:], k_ps[:], Copy)
                        # squares for the l2 norm (Pool, SBUF-only)
                        sq = sbA.tile([128, DIM], F16, tag="sq")
                        nc.gpsimd.tensor_mul(sq[:], k_sb[:], k_sb[:])

                        cosb = (
                            cosC_t[:, ct * HD : (ct + 1) * HD]
                            .unsqueeze(1)
                            .broadcast_to([128, H, HD])
                        )
                        sinb4 = (
                            sinC_t[:, ct * HD : (ct + 1) * HD]
                            .rearrange("p (g two) -> p g two", two=2)
                            .unsqueeze(1)
                            .broadcast_to([128, H, HD // 2, 2])
                        )
                        k3 = k_sb[:].rearrange("p (h f) -> p h f", h=H)
                        k_sw = k_sb[:].rearrange(
                            "p (h g two) -> p h g two", h=H, two=2
                        )[:, :, :, ::-1]

                        m1 = sb1.tile([128, DIM], F16, tag="m1")
                        nc.vector.tensor_tensor(
                            m1[:].rearrange("p (h f) -> p h f", h=H), k3, cosb, MUL
                        )
                        red = smA.tile([128, H], F32, tag="red")
                        nc.vector.tensor_reduce(
                            red[:],
                            sq[:].rearrange("p (h f) -> p h f", h=H),
                            mybir.AxisListType.X,
                            ADD,
                        )
                        srt = smA.tile([128, H], F32, tag="srt")
                        nc.scalar.activation(srt[:], red[:], Sqrt)
                        rs = smA.tile([128, H], F32, tag="rs")
                        nc.vector.reciprocal(rs[:], srt[:])
                        rsm = smA.tile([128, H], F16, tag="rsm")
                        nc.vector.tensor_scalar_mul(
                            rsm[:], rs[:], maskC_t[:, ct : ct + 1]
                        )
                        m2 = sb1.tile([128, DIM], F16, tag="m2")
                        nc.vector.tensor_tensor(
                            m2[:].rearrange("p (h g two) -> p h g two", h=H, two=2),
                            k_sw,
                            sinb4,
                            MUL,
                        )
                        s = sb1.tile([128, DIM], F16, tag="s")
                        nc.gpsimd.tensor_tensor(s[:], m1[:], m2[:], ADD)
                        khat = sbA.tile([128, DIM], F16, tag="khat")
                        rsb = rsm[:].unsqueeze(2).broadcast_to([128, H, HD])
                        nc.vector.tensor_tensor(
                            khat[:].rearrange("p (h f) -> p h f", h=H),
                            s[:].rearrange("p (h f) -> p h f", h=H),
                            rsb,
                            MUL,
                        )

                        # kvT Grams are issued one iteration late (software
                        # pipelining) so PE never waits on the khat chain
                        kv_pending.append((ct, khat, v_sb))
                        if len(kv_pending) > 1:
                            _emit_kv(kv_pending.pop(0))

                    # leave the final c-tile's Grams pending: they wait on
                    # the last khat chain (~4us of vector latency), so the
                    # first two q projections are emitted ahead of them to
                    # keep PE fed (PE is in-order).
                    kv_tail = [kv_pending.pop(0)] if kv_pending else []

                    def _emit_kv_tail():
                        while kv_tail:
                            _emit_kv(kv_tail.pop(0))

                    if haveB:
                        qall_t = qSP.tile([128, NJ * 512], F16, tag="qall", name="qall")
                        qall_cur[0] = qall_t
                        p1_early = [_emit_pass1_jt(0, jt) for jt in range(2)]
                    else:
                        p1_early = []
                    _emit_kv_tail()

                # ==== Fused phase B+C: q proj/norm/rope + (attn·Wo) ===
                with ExitStack() as ctxB:
                  if "B" in phases and "C" in phases:
                    psN = ctxB.enter_context(
                        tc.tile_pool(name="psN", bufs=1, space="PSUM")
                    )
                    psR = ctxB.enter_context(
                        tc.tile_pool(name="psR", bufs=1, space="PSUM")
                    )
                    psO = ctxB.enter_context(
                        tc.tile_pool(name="psO", bufs=3, space="PSUM")
                    )
                    sbB = ctxB.enter_context(tc.tile_pool(name="sbB", bufs=3))
                    sbQ = ctxB.enter_context(
                        tc.tile_pool(name="sbQ", bufs=NQ_T)
                    )

                    def _emit_pass1(ct_, early=()):
                        """q proj + squares + norm accumulation; returns
                        (qall, norms_ps).  Norms matmuls interleave with the
                        projection chains (already-emitted `early` j-tiles get
                        their norms matmul first)."""
                        if not early:
                            qall_t2 = qSP.tile(
                                [128, NJ * 512], F16, tag="qall", name="qall"
                            )
                            qall_cur[0] = qall_t2
                        pairs = list(early)
                        norms_ps = psN.tile([16, 512], F32, tag="norms")
                        for jt in range(NJ):
                            if jt >= len(pairs):
                                pairs.append(_emit_pass1_jt(ct_, jt))
                            nc.tensor.matmul(
                                norms_ps[:],
                                ind16T_t[:, jt * 16 : (jt + 1) * 16],
                                pairs[jt][1][:],
                                start=(jt == 0),
                                stop=(jt == NJ - 1),
                            )
                        return qall_cur[0], norms_ps

                    def _emit_pass2(ct_, qall, norms_ps):
                        """rsqrt + rope + norm scale; rot/rep matmul results
                        are consumed straight from PSUM (no evictions).
                        Returns qh_all."""
                        cs = slice(ct_ * 512, (ct_ + 1) * 512)
                        nsrt = sbB.tile([16, 512], F32, tag="nsrt")
                        nc.scalar.activation(nsrt[:], norms_ps[:], Sqrt)
                        nrcp = sbB.tile([16, 512], F32, tag="nrcp")
                        nc.vector.reciprocal(nrcp[:], nsrt[:])
                        rs16 = sbB.tile([16, 512], F16, tag="rs16")
                        nc.gpsimd.tensor_copy(rs16[:], nrcp[:])

                        qh_all = sbQ.tile([128, NJ * 512], F16, tag="qhall")
                        s_tiles = []
                        for jt in range(NJ):
                            sl = slice(jt * 512, (jt + 1) * 512)
                            rot_ps = psR.tile([128, 512], F32, tag="rot")
                            nc.tensor.matmul(
                                rot_ps[:],
                                P_t[:],
                                qall[:, sl],
                                start=True,
                                stop=True,
                            )
                            t1 = sbB.tile([128, 512], F16, tag="t1")
                            nc.vector.tensor_tensor(
                                t1[:], qall[:, sl], cosF_t[:, cs], MUL
                            )
                            t2 = sbB.tile([128, 512], F16, tag="t2")
                            nc.vector.tensor_tensor(
                                t2[:], rot_ps[:], sinF_t[:, cs], MUL
                            )
                            s = sqP.tile([128, 512], F16, tag="sB")
                            nc.gpsimd.tensor_tensor(s[:], t1[:], t2[:], ADD)
                            s_tiles.append(s)
                        # rep matmuls second: they need rs16, whose chain
                        # overlaps the rot/rope work above
                        for jt in range(NJ):
                            rep_ps = psR.tile([128, 512], F32, tag="rep")
                            nc.tensor.matmul(
                                rep_ps[:],
                                ind16_t[:, jt * 128 : (jt + 1) * 128],
                                rs16[:],
                                start=True,
                                stop=True,
                            )
                            nc.vector.tensor_tensor(
                                qh_all[:, jt * 512 : (jt + 1) * 512],
                                s_tiles[jt][:],
                                rep_ps[:],
                                MUL,
                            )
                        return qh_all

                    def _emit_G():
                        """kvT load (blocks on collective) -> G = kvT^T@WoT."""
                        kvpool = ctxB.enter_context(
                            tc.tile_pool(name="kvpool", bufs=1)
                        )
                        kvf0 = kvpool.tile([128, NPAIR * 128], F16, tag="kvf0")
                        nc.sync.dma_start(out=kvf0[:], in_=kv_out_d[0].ap())
                        kvf1 = kvpool.tile([128, NPAIR * 128], F16, tag="kvf1")
                        nc.sync.dma_start(out=kvf1[:], in_=kv_out_d[1].ap())
                        kvf = kvpool.tile([128, NPAIR * 128], F16, tag="kvf")
                        nc.vector.tensor_tensor(kvf[:], kvf0[:], kvf1[:], ADD)
                        kvblk = kvpool.tile([128, NPAIR * 128], F16, tag="kvblk")
                        nc.vector.memset(kvblk[:], 0.0)
                        nc.vector.tensor_copy(
                            kvblk[0:64, :].rearrange(
                                "p (t f) -> p t f", t=NPAIR
                            )[:, :, 0:64],
                            kvf[0:64, :].rearrange("p (t f) -> p t f", t=NPAIR)[
                                :, :, 0:64
                            ],
                        )
                        nc.vector.tensor_copy(
                            kvblk[64:128, :].rearrange(
                                "p (t f) -> p t f", t=NPAIR
                            )[:, :, 64:128],
                            kvf[64:128, :].rearrange(
                                "p (t f) -> p t f", t=NPAIR
                            )[:, :, 64:128],
                        )
                        for half in range(2):
                            for hp in range(NPAIR):
                                fs = slice(hp * DIM + half * 512,
                                           hp * DIM + (half + 1) * 512)
                                g_ps = psO.tile([128, 512], F32, tag="o_ps")
                                nc.tensor.matmul(
                                    g_ps[:],
                                    kvblk[:, hp * 128 : (hp + 1) * 128],
                                    wo_all[:, fs],
                                    start=True,
                                    stop=True,
                                )
                                nc.scalar.activation(
                                    G_all[:, fs], g_ps[:], Copy
                                )

                    def _emit_out(ct_, qh_):
                        cs_ = slice(ct_ * 512, (ct_ + 1) * 512)
                        o_all = sbQ.tile([128, NJ * 512], F16, tag="o_all")
                        outv = out_d.rearrange("(t p) c -> p t c", p=128)
                        for et in range(NJ):
                            elo = et * 128
                            o_ps = psO.tile([128, 512], F32, tag="o_ps")
                            for jt in range(NJ):
                                nc.tensor.matmul(
                                    o_ps[:],
                                    G_all[
                                        :, jt * DIM + elo : jt * DIM + elo + 128
                                    ],
                                    qh_[:, jt * 512 : (jt + 1) * 512],
                                    start=(jt == 0),
                                    stop=(jt == NJ - 1),
                                )
                            nc.scalar.activation(
                                o_all[:, et * 512 : (et + 1) * 512], o_ps[:], Copy
                            )
                            # per-et DMA so the tail shrinks to one et
                            nc.sync.dma_start(
                                out=outv[:, et, cs_],
                                in_=o_all[:, et * 512 : (et + 1) * 512],
                            )

                    # emission order: pass1(0), pass2(0), pass1(1), pass2(1),
                    # G, out(0), out(1) — G sits behind the full q pipeline
                    # of independent PE work to hide the AllReduce latency.
                    q0, n0 = _emit_pass1(0, early=p1_early)
                    qh0 = _emit_pass2(0, q0, n0)
                    q1, n1 = _emit_pass1(1)
                    qh1 = _emit_pass2(1, q1, n1)
                    _emit_G()
                    _emit_out(0, qh0)
                    _emit_out(1, qh1)

    nc.compile()
    return nc


_NC_CACHE = None


def _get_nc():
    global _NC_CACHE
    if _NC_CACHE is None:
        _NC_CACHE = build_nc()
    return _NC_CACHE


def _plan_rows(mask):
    """Split each batch's unmasked row indices over its 4 cores.
    Returns rows[core] = np.ndarray of original row ids (len <= R)."""
    mask = np.asarray(mask)
    rows = []
    for b in range(B):
        idx = np.flatnonzero(mask[b] != 0)
        n = len(idx)
        assert n <= 4 * R, f"unmasked rows {n} exceed capacity {4 * R}"
        szs = [n // 4 + (1 if i < n % 4 else 0) for i in range(4)]
        off = 0
        for cc in range(4):
            rows.append(idx[off : off + szs[cc]])
            off += szs[cc]
    return rows


def make_in_maps(x, mask, Wq, Wk, Wv, Wo, norm_const):
    x = np.asarray(x, np.float32)
    mask = np.asarray(mask)
    Wq = np.asarray(Wq, np.float32)
    Wk = np.asarray(Wk, np.float32)
    Wv = np.asarray(Wv, np.float32)
    Wo = np.asarray(Wo, np.float32)
    norm_const = np.asarray(norm_const, np.float32).reshape(H)

    sig = 1.0 / (1.0 + np.exp(-norm_const.astype(np.float64)))
    svec = np.float64(C) ** (-sig)  # [H]
    s_cols = np.repeat(svec, HD)  # [DIM]

    f16 = np.float16
    WkT = np.ascontiguousarray(Wk.T).astype(f16)
    WvT = np.ascontiguousarray((Wv * s_cols[:, None].astype(np.float32)).T).astype(
        f16
    )
    WqT = np.ascontiguousarray(Wq.T).astype(f16)
    WoT = np.ascontiguousarray(Wo.T).astype(f16)

    inv_freq = 1.0 / (
        ROPE_THETA ** (np.arange(0, HD, 2, dtype=np.float64) / HD)
    )  # [32]
    freq_of_j = np.repeat(inv_freq, 2)  # [64] interleaved

    ind16T = np.zeros((DIM, 16), f16)
    for jt in range(NJ):
        for kk in range(128):
            ind16T[jt * 128 + kk, 2 * jt + (kk >= 64)] = 1.0

    ind16 = np.zeros((16, DIM), f16)
    for jt in range(NJ):
        for m in range(128):
            ind16[2 * jt + (m >= 64), jt * 128 + m] = 1.0

    Pmat = np.zeros((128, 128), f16)
    for i in range(64):
        Pmat[2 * i + 1, 2 * i] = -1.0  # out[2i] = -q[2i+1]
        Pmat[2 * i, 2 * i + 1] = 1.0  # out[2i+1] = q[2i]

    rows_per_core = _plan_rows(mask)

    in_maps = []
    for core in range(N_CORES):
        b = core // (N_CORES // B)
        rows = rows_per_core[core]
        sz = len(rows)
        pos = np.zeros(R, np.float64)
        pos[:sz] = rows.astype(np.float64)

        # gathered x columns; pads filled with 1.0 (khat row is zeroed by
        # the mask so pads contribute nothing)
        xTc = np.ones((DIM, R), f16)
        xTc[:, :sz] = x[b, rows, :].T.astype(f16)

        angC = pos[:, None] * freq_of_j[None, :]  # [R, 64]
        cosCc = np.cos(angC).astype(f16)
        sinCc = np.sin(angC).astype(np.float32)
        # sign fold for the swap formulation: even j -> -sin, odd j -> +sin
        sinCc[:, 0::2] *= -1.0
        sinCc = sinCc.astype(f16)

        angF = freq_of_j[:, None] * pos[None, :]  # [64, R]
        angF2 = np.concatenate([angF, angF], axis=0)  # [128, R]
        cosFc = np.cos(angF2).astype(f16)
        sinFc = np.sin(angF2).astype(f16)

        mrow = np.zeros(R, np.float32)
        mrow[:sz] = 1.0
        maskCc = np.ascontiguousarray(mrow.reshape(NC_T, 128).T)  # [128, NC_T]

        in_maps.append(
            {
                "xT": xTc,
                "WkT": WkT,
                "WvT": WvT,
                "WqT": WqT,
                "WoT": WoT,
                "cosC": cosCc,
                "sinC": sinCc,
                "cosF": cosFc,
                "sinF": sinFc,
                "maskC": maskCc,
                "ind16T": ind16T,
                "ind16": ind16,
                "Pmat": Pmat,
            }
        )
    return in_maps


def assemble_output(results, mask):
    rows_per_core = _plan_rows(mask)
    out = np.zeros((B, C, DIM), np.float32)
    for core in range(N_CORES):
        b = core // (N_CORES // B)
        rows = rows_per_core[core]
        sz = len(rows)
        out[b, rows, :] = results[core]["out"].T[:sz].astype(np.float32)
    return out


def kernel(x, mask, Wq, Wk, Wv, Wo, norm_const):
    nc = _get_nc()
    in_maps = make_in_maps(x, mask, Wq, Wk, Wv, Wo, norm_const)
    res = run_bass_kernel_spmd(nc, in_maps, list(range(N_CORES)))
    return assemble_output(res.results, mask)


# revision 85
# speedup vs baseline: 1.2986x; 1.2986x over previous
"""Trainium2 Bass kernel for nn_Attention_43413529428606 (linear attention
with l2-normed q/k, interleaved RoPE, mask, per-head power scaling).

v2: mask compaction.  Only the ~50% unmasked rows are processed: the host
gathers rows with mask==1 per batch, splits them over the 4 cores of that
batch (cores 0-3 batch 0, 4-7 batch 1; capacity 1024 rows/core), and
scatters the output back (masked rows are zero by construction).  Each core
projects k/v for its rows, applies l2norm+RoPE, accumulates the per-head
transposed Gram state kvT = sum_c v ⊗ khat, AllReduces it in fp16 (256 KB)
within its batch group, folds Wo into the state (G = kvT^T @ WoT per head
pair), then computes q/norm/rope and the fused attn+out projection
out = qhat @ G for its rows.  Activation-engine usage is restricted to
{Copy, Sqrt} (one table set; 1/sqrt runs as Act-Sqrt + DVE-reciprocal),
squares and sums run on DVE/Pool, and the load DMAs are ordered so the
first projection starts after ~1.5 MiB.  The two AllReduce halves overlap
the second half of phase A and the whole q pipeline.

Self-contained: hardcodes all shapes; no sibling imports.
"""

import sys

for _p in ("/opt/trn_rl_repo",):
    if _p not in sys.path:
        sys.path.append(_p)

from contextlib import ExitStack

import numpy as np

import concourse.bass as bass
import concourse.bacc as bacc
import concourse.tile as tile
from concourse import mybir
from concourse.bass_utils import run_bass_kernel_spmd

F32 = mybir.dt.float32
F16 = mybir.dt.float16

DIM = 1024
H = 16
HD = 64
B = 2
C = 8192
ROPE_THETA = 10000.0

N_CORES = 8
R = 1024  # compacted rows per core (capacity; ~n_unmasked/4)
NC_T = R // 128  # 8 c-tiles of 128 (phase A)
NQ_T = R // 512  # 2 c-supertiles of 512 (phase B)
ND = DIM // 128  # 8 d-chunks
NJ = DIM // 128  # 8 j-tiles
NPAIR = H // 2  # 8 head pairs

Copy = mybir.ActivationFunctionType.Copy
Sqrt = mybir.ActivationFunctionType.Sqrt
MUL = mybir.AluOpType.mult
ADD = mybir.AluOpType.add


def build_nc(sim_mode=False, phases="ABC", reps=1, coll=True):
    nc = bacc.Bacc(
        "TRN2",
        target_bir_lowering=False,
        debug=False,
        num_devices=1 if sim_mode else N_CORES,
    )

    # ---- DRAM parameters (per-core shapes, fp16 data path) ----
    xT = nc.dram_tensor("xT", [DIM, R], F16, kind="ExternalInput").ap()
    WkT = nc.dram_tensor("WkT", [DIM, DIM], F16, kind="ExternalInput").ap()
    WvT = nc.dram_tensor("WvT", [DIM, DIM], F16, kind="ExternalInput").ap()
    WqT = nc.dram_tensor("WqT", [DIM, DIM], F16, kind="ExternalInput").ap()
    WoT = nc.dram_tensor("WoT", [DIM, DIM], F16, kind="ExternalInput").ap()
    cosC = nc.dram_tensor("cosC", [R, HD], F16, kind="ExternalInput").ap()
    sinC = nc.dram_tensor("sinC", [R, HD], F16, kind="ExternalInput").ap()
    cosF = nc.dram_tensor("cosF", [128, R], F16, kind="ExternalInput").ap()
    sinF = nc.dram_tensor("sinF", [128, R], F16, kind="ExternalInput").ap()
    maskC = nc.dram_tensor("maskC", [128, NC_T], F32, kind="ExternalInput").ap()
    ind16T = nc.dram_tensor("ind16T", [DIM, 16], F16, kind="ExternalInput").ap()
    ind16 = nc.dram_tensor("ind16", [16, DIM], F16, kind="ExternalInput").ap()
    Pmat = nc.dram_tensor("Pmat", [128, 128], F16, kind="ExternalInput").ap()

    kv_in_d = [
        nc.dram_tensor(f"kv_in_d{h}", [128, NPAIR * 128], F16) for h in range(2)
    ]
    kv_out_d = [
        nc.dram_tensor(f"kv_out_d{h}", [128, NPAIR * 128], F16) for h in range(2)
    ]

    out_d = nc.dram_tensor("out", [DIM, R], F16, kind="ExternalOutput").ap()

    def blkview(dram_ap, csl):
        return dram_ap.rearrange("(t p) c -> p t c", p=128)[:, :, csl]

    with tile.TileContext(nc) as tc:
        with ExitStack() as ctx:
            consts = ctx.enter_context(tc.tile_pool(name="consts", bufs=1))

            cosC_t = consts.tile([128, NC_T * HD], F16, tag="cosC")
            sinC_t = consts.tile([128, NC_T * HD], F16, tag="sinC")
            maskC_t = consts.tile([128, NC_T], F32, tag="maskC")
            ind16T_t = consts.tile([128, NJ * 16], F16, tag="ind16T")
            ind16_t = consts.tile([16, DIM], F16, tag="ind16")
            P_t = consts.tile([128, 128], F16, tag="Pmat")
            consts_emitted = [False]

            def _emit_consts():
                if consts_emitted[0]:
                    return
                consts_emitted[0] = True
                nc.sync.dma_start(
                    out=cosC_t[:].rearrange("p (t f) -> p t f", t=NC_T),
                    in_=cosC[:].rearrange("(t p) f -> p t f", p=128),
                )
                nc.sync.dma_start(
                    out=sinC_t[:].rearrange("p (t f) -> p t f", t=NC_T),
                    in_=sinC[:].rearrange("(t p) f -> p t f", p=128),
                )
                nc.sync.dma_start(out=maskC_t[:], in_=maskC[:])
                nc.sync.dma_start(
                    out=ind16T_t[:].rearrange("p (t f) -> p t f", t=NJ),
                    in_=ind16T[:].rearrange("(t p) f -> p t f", p=128),
                )
                nc.sync.dma_start(out=ind16_t[:], in_=ind16[:])
                nc.sync.dma_start(out=P_t[:], in_=Pmat[:])

            for _rep in range(reps):
              with ExitStack() as ctxX:
                xpool = ctxX.enter_context(tc.tile_pool(name="xpool", bufs=1))
                xT_all = xpool.tile([128, ND * R], F16, tag="xT")

                def _xdma_c(lo, hi, eng=None):
                    csl = slice(lo, hi)
                    (eng or nc.sync).dma_start(
                        out=xT_all[:]
                        .rearrange("p (t c) -> p t c", t=ND)[:, :, csl],
                        in_=xT[:, csl].rearrange("(t p) c -> p t c", p=128),
                    )

                def _xdma(xc, eng=None):
                    _xdma_c(xc * 256, (xc + 1) * 256, eng)

                def _wdma_j(wt, wsrc, lo, hi, eng=None):
                    jsl = slice(lo, hi)
                    (eng or nc.sync).dma_start(
                        out=wt[:].rearrange("p (t f) -> p t f", t=ND)[:, :, jsl],
                        in_=wsrc[:].rearrange("(t p) f -> p t f", p=128)[
                            :, :, jsl
                        ],
                    )

                def _wdma(wt, wsrc, jc, eng=None):
                    _wdma_j(wt, wsrc, jc * 512, (jc + 1) * 512, eng)

                def xsl(dc, csl):
                    lo = dc * R
                    return xT_all[:, lo + csl.start : lo + csl.stop]

                if "B" in phases and "C" in phases:
                    wBC = ctxX.enter_context(tc.tile_pool(name="wBC", bufs=1))
                    wq_all = wBC.tile([128, ND * DIM], F16, tag="wq")
                    wo_all = wBC.tile([128, ND * DIM], F16, tag="wo")
                    cosF_t = wBC.tile([128, R], F16, tag="cosF")
                    sinF_t = wBC.tile([128, R], F16, tag="sinF")
                    G_pool = ctxX.enter_context(
                        tc.tile_pool(name="G_pool", bufs=1)
                    )
                    G_all = G_pool.tile([128, NJ * DIM], F16, tag="G")

                def _emit_phaseB_loads():
                    if not ("B" in phases and "C" in phases):
                        return
                    nc.sync.dma_start(
                        out=wq_all[:].rearrange("p (t f) -> p t f", t=ND),
                        in_=WqT[:].rearrange("(t p) f -> p t f", p=128),
                    )
                    nc.sync.dma_start(
                        out=wo_all[:].rearrange("p (t f) -> p t f", t=ND),
                        in_=WoT[:].rearrange("(t p) f -> p t f", p=128),
                    )
                    nc.sync.dma_start(out=cosF_t[:], in_=cosF[:])
                    nc.sync.dma_start(out=sinF_t[:], in_=sinF[:])

                # pools shared between late phase A and pass1 of phase B:
                # psB gets PSUM banks 0-1, ahead of psA, so the first q
                # projections can overlap the phase-A Gram tail
                haveB = "B" in phases and "C" in phases
                ctxO = ctxX.enter_context(ExitStack())
                if haveB:
                    psB = ctxO.enter_context(
                        tc.tile_pool(name="psB", bufs=2, space="PSUM")
                    )
                    qSP = ctxO.enter_context(tc.tile_pool(name="qSP", bufs=2))
                    sqP = ctxO.enter_context(
                        tc.tile_pool(name="sqP", bufs=NJ + 1)
                    )
                    # current supertile's contiguous q buffer (pass1 evicts
                    # into slices; one DMA pair-swaps the whole thing)
                    qall_cur = [None]

                    def _emit_pass1_jt(ct_, jt):
                        """q proj + eviction + square for one j-tile."""
                        cs = slice(ct_ * 512, (ct_ + 1) * 512)
                        jlo = jt * 128
                        q_ps = psB.tile([128, 512], F32, tag="q_ps")
                        for dc in range(ND):
                            nc.tensor.matmul(
                                q_ps[:],
                                wq_all[
                                    :, dc * DIM + jlo : dc * DIM + jlo + 128
                                ],
                                xsl(dc, cs),
                                start=(dc == 0),
                                stop=(dc == ND - 1),
                            )
                        q_sb = qall_cur[0][:, jt * 512 : (jt + 1) * 512]
                        nc.scalar.activation(q_sb, q_ps[:], Copy)
                        sq = sqP.tile([128, 512], F16, tag="sqB")
                        nc.vector.tensor_mul(sq[:], q_sb, q_sb)
                        return q_sb, sq

                # ========= Phase A: k/v proj + process + kvT Grams ==========
                with ExitStack() as ctxA:
                  if "A" not in phases:
                    _emit_consts()
                    for xc in range(4):
                        _xdma(xc)
                    _emit_phaseB_loads()
                    p1_early = []

                    def _emit_kv_tail():
                        pass
                  else:
                    wA = ctxA.enter_context(tc.tile_pool(name="wA", bufs=1))
                    psA = ctxA.enter_context(
                        tc.tile_pool(name="psA", bufs=2, space="PSUM")
                    )
                    pskv = ctxA.enter_context(
                        tc.tile_pool(name="pskv", bufs=1, space="PSUM")
                    )
                    sbA = ctxA.enter_context(tc.tile_pool(name="sbA", bufs=2))
                    sb1 = ctxA.enter_context(tc.tile_pool(name="sb1", bufs=2))
                    smA = ctxA.enter_context(tc.tile_pool(name="smA", bufs=2))

                    # explicit load order on the SP dma queue: everything the
                    # phase-A pipeline needs, j/c-chunked so compute starts
                    # after ~1.5 MiB, then the phase-B weights
                    wk_all = wA.tile([128, ND * DIM], F16, tag="wk")
                    wv_all = wA.tile([128, ND * DIM], F16, tag="wv")
                    # first load on the idle Act queue (shorter preamble
                    # than SP), rest on SP in priority order
                    _wdma(wk_all, WkT, 0, eng=nc.scalar)
                    _xdma(0)
                    _wdma(wv_all, WvT, 0)
                    _wdma(wk_all, WkT, 1)
                    _xdma(1)
                    _emit_consts()
                    _wdma(wv_all, WvT, 1)
                    _xdma(2)
                    _xdma(3)
                    _emit_phaseB_loads()

                    kv_ps = pskv.tile([128, NPAIR * 128], F32, tag="kvps")
                    kv_pending = []
                    HALF_T = NC_T // 2

                    # kvT Gram: out[e,d] = sum_c v[c,e] khat[c,d].
                    # Accumulated in two c-halves; each half is AllReduced
                    # separately so the first collective overlaps the second
                    # half of phase A.  On HW start=True zeroes the whole
                    # PSUM bank, so only the first pair written to each bank
                    # per half may carry it.
                    def _emit_kv(item):
                        ct_, khat_, v_ = item
                        first = ct_ % HALF_T == 0
                        last = ct_ % HALF_T == HALF_T - 1
                        for p in range(NPAIR):
                            ps_ = slice(p * 128, (p + 1) * 128)
                            nc.tensor.matmul(
                                kv_ps[:, ps_],
                                v_[:, ps_],
                                khat_[:, ps_],
                                start=(
                                    True if sim_mode else (first and p % 4 == 0)
                                ),
                                stop=(True if sim_mode else last),
                            )
                        if last:
                            _emit_kv_flush(ct_ // HALF_T)

                    def _emit_kv_flush(h):
                        kv_sb = sbA.tile([128, NPAIR * 128], F16, tag="kv_sb")
                        nc.vector.tensor_copy(kv_sb[:], kv_ps[:])
                        nc.sync.dma_start(out=kv_in_d[h].ap(), in_=kv_sb[:])
                        if sim_mode or not coll:
                            nc.sync.dma_start(
                                out=kv_out_d[h].ap(), in_=kv_in_d[h].ap()
                            )
                        else:
                            nc.gpsimd.collective_compute(
                                "AllReduce",
                                ADD,
                                replica_groups=[[0, 1, 2, 3], [4, 5, 6, 7]],
                                ins=[kv_in_d[h].ap().opt()],
                                outs=[kv_out_d[h].ap().opt()],
                            )

                    for ct in range(NC_T):
                        cs = slice(ct * 128, (ct + 1) * 128)
                        k_ps = psA.tile([128, DIM], F32, tag="proj_ps")
                        v_ps = psA.tile([128, DIM], F32, tag="proj_ps")
                        halves = [(0, 512, True), (512, 1024, True)]
                        for jlo, jhi, first in halves:
                            js = slice(jlo, jhi)
                            for dc in range(ND):
                                nc.tensor.matmul(
                                    k_ps[:, js],
                                    xsl(dc, cs),
                                    wk_all[
                                        :, dc * DIM + js.start : dc * DIM + js.stop
                                    ],
                                    start=(dc == 0 and first),
                                    stop=(dc == ND - 1),
                                )
                            for dc in range(ND):
                                nc.tensor.matmul(
                                    v_ps[:, js],
                                    xsl(dc, cs),
                                    wv_all[
                                        :, dc * DIM + js.start : dc * DIM + js.stop
                                    ],
                                    start=(dc == 0 and first),
                                    stop=(dc == ND - 1),
                                )

                        # evictions: v and k on Act (cast fp16)
                        v_sb = sbA.tile([128, DIM], F16, tag="v_sb")
                        nc.scalar.activation(v_sb[:], v_ps[:], Copy)
                        k_sb = sbA.tile([128, DIM], F16, tag="k_sb")
                        nc.scalar.activation(k_sb[:], k_ps[:], Copy)
                        # squares for the l2 norm (Pool, SBUF-only)
                        sq = sbA.tile([128, DIM], F16, tag="sq")
                        nc.gpsimd.tensor_mul(sq[:], k_sb[:], k_sb[:])

                        cosb = (
                            cosC_t[:, ct * HD : (ct + 1) * HD]
                            .unsqueeze(1)
                            .broadcast_to([128, H, HD])
                        )
                        sinb4 = (
                            sinC_t[:, ct * HD : (ct + 1) * HD]
                            .rearrange("p (g two) -> p g two", two=2)
                            .unsqueeze(1)
                            .broadcast_to([128, H, HD // 2, 2])
                        )
                        k3 = k_sb[:].rearrange("p (h f) -> p h f", h=H)
                        k_sw = k_sb[:].rearrange(
                            "p (h g two) -> p h g two", h=H, two=2
                        )[:, :, :, ::-1]

                        m1 = sb1.tile([128, DIM], F16, tag="m1")
                        nc.vector.tensor_tensor(
                            m1[:].rearrange("p (h f) -> p h f", h=H), k3, cosb, MUL
                        )
                        red = smA.tile([128, H], F32, tag="red")
                        nc.vector.tensor_reduce(
                            red[:],
                            sq[:].rearrange("p (h f) -> p h f", h=H),
                            mybir.AxisListType.X,
                            ADD,
                        )
                        srt = smA.tile([128, H], F32, tag="srt")
                        nc.scalar.activation(srt[:], red[:], Sqrt)
                        rs = smA.tile([128, H], F32, tag="rs")
                        nc.vector.reciprocal(rs[:], srt[:])
                        rsm = smA.tile([128, H], F16, tag="rsm")
                        nc.vector.tensor_scalar_mul(
                            rsm[:], rs[:], maskC_t[:, ct : ct + 1]
                        )
                        m2 = sb1.tile([128, DIM], F16, tag="m2")
                        nc.vector.tensor_tensor(
                            m2[:].rearrange("p (h g two) -> p h g two", h=H, two=2),
                            k_sw,
                            sinb4,
                            MUL,
                        )
                        s = sb1.tile([128, DIM], F16, tag="s")
                        nc.gpsimd.tensor_tensor(s[:], m1[:], m2[:], ADD)
                        khat = sbA.tile([128, DIM], F16, tag="khat")
                        rsb = rsm[:].unsqueeze(2).broadcast_to([128, H, HD])
                        nc.vector.tensor_tensor(
                            khat[:].rearrange("p (h f) -> p h f", h=H),
                            s[:].rearrange("p (h f) -> p h f", h=H),
                            rsb,
                            MUL,
                        )

                        # kvT Grams are issued one iteration late (software
                        # pipelining) so PE never waits on the khat chain
                        kv_pending.append((ct, khat, v_sb))
                        if len(kv_pending) > 1:
                            _emit_kv(kv_pending.pop(0))

                    # leave the final c-tile's Grams pending: they wait on
                    # the last khat chain (~4us of vector latency), so the
                    # first two q projections are emitted ahead of them to
                    # keep PE fed (PE is in-order).
                    kv_tail = [kv_pending.pop(0)] if kv_pending else []

                    def _emit_kv_tail():
                        while kv_tail:
                            _emit_kv(kv_tail.pop(0))

                    if haveB:
                        qall_t = qSP.tile([128, NJ * 512], F16, tag="qall", name="qall")
                        qall_cur[0] = qall_t
                        p1_early = [_emit_pass1_jt(0, jt) for jt in range(2)]
                    else:
                        p1_early = []
                    _emit_kv_tail()

                # ==== Fused phase B+C: q proj/norm/rope + (attn·Wo) ===
                with ExitStack() as ctxB:
                  if "B" in phases and "C" in phases:
                    psN = ctxB.enter_context(
                        tc.tile_pool(name="psN", bufs=1, space="PSUM")
                    )
                    psR = ctxB.enter_context(
                        tc.tile_pool(name="psR", bufs=1, space="PSUM")
                    )
                    psO = ctxB.enter_context(
                        tc.tile_pool(name="psO", bufs=3, space="PSUM")
                    )
                    sbB = ctxB.enter_context(tc.tile_pool(name="sbB", bufs=3))
                    sbQ = ctxB.enter_context(
                        tc.tile_pool(name="sbQ", bufs=NQ_T)
                    )

                    def _emit_pass1(ct_, early=()):
                        """q proj + squares + norm accumulation; returns
                        (qall, norms_ps).  Norms matmuls interleave with the
                        projection chains (already-emitted `early` j-tiles get
                        their norms matmul first)."""
                        if not early:
                            qall_t2 = qSP.tile(
                                [128, NJ * 512], F16, tag="qall", name="qall"
                            )
                            qall_cur[0] = qall_t2
                        pairs = list(early)
                        norms_ps = psN.tile([16, 512], F32, tag="norms")
                        for jt in range(NJ):
                            if jt >= len(pairs):
                                pairs.append(_emit_pass1_jt(ct_, jt))
                            nc.tensor.matmul(
                                norms_ps[:],
                                ind16T_t[:, jt * 16 : (jt + 1) * 16],
                                pairs[jt][1][:],
                                start=(jt == 0),
                                stop=(jt == NJ - 1),
                            )
                        return qall_cur[0], norms_ps

                    def _emit_pass2(ct_, qall, norms_ps):
                        """rsqrt + rope + norm scale; rot/rep matmul results
                        are consumed straight from PSUM (no evictions).
                        Returns qh_all."""
                        cs = slice(ct_ * 512, (ct_ + 1) * 512)
                        nsrt = sbB.tile([16, 512], F32, tag="nsrt")
                        nc.scalar.activation(nsrt[:], norms_ps[:], Sqrt)
                        nrcp = sbB.tile([16, 512], F32, tag="nrcp")
                        nc.vector.reciprocal(nrcp[:], nsrt[:])
                        rs16 = sbB.tile([16, 512], F16, tag="rs16")
                        nc.gpsimd.tensor_copy(rs16[:], nrcp[:])

                        qh_all = sbQ.tile([128, NJ * 512], F16, tag="qhall")
                        s_tiles = []
                        for jt in range(NJ):
                            sl = slice(jt * 512, (jt + 1) * 512)
                            rot_ps = psR.tile([128, 512], F32, tag="rot")
                            nc.tensor.matmul(
                                rot_ps[:],
                                P_t[:],
                                qall[:, sl],
                                start=True,
                                stop=True,
                            )
                            t1 = sbB.tile([128, 512], F16, tag="t1")
                            nc.vector.tensor_tensor(
                                t1[:], qall[:, sl], cosF_t[:, cs], MUL
                            )
                            t2 = sbB.tile([128, 512], F16, tag="t2")
                            nc.vector.tensor_tensor(
                                t2[:], rot_ps[:], sinF_t[:, cs], MUL
                            )
                            s = sqP.tile([128, 512], F16, tag="sB")
                            nc.gpsimd.tensor_tensor(s[:], t1[:], t2[:], ADD)
                            s_tiles.append(s)
                        # rep matmuls second: they need rs16, whose chain
                        # overlaps the rot/rope work above
                        for jt in range(NJ):
                            rep_ps = psR.tile([128, 512], F32, tag="rep")
                            nc.tensor.matmul(
                                rep_ps[:],
                                ind16_t[:, jt * 128 : (jt + 1) * 128],
                                rs16[:],
                                start=True,
                                stop=True,
                            )
                            nc.vector.tensor_tensor(
                                qh_all[:, jt * 512 : (jt + 1) * 512],
                                s_tiles[jt][:],
                                rep_ps[:],
                                MUL,
                            )
                        return qh_all

                    def _emit_G():
                        """kvT load (blocks on collective) -> G = kvT^T@WoT."""
                        kvpool = ctxB.enter_context(
                            tc.tile_pool(name="kvpool", bufs=1)
                        )
                        kvf0 = kvpool.tile([128, NPAIR * 128], F16, tag="kvf0")
                        nc.sync.dma_start(out=kvf0[:], in_=kv_out_d[0].ap())
                        kvf1 = kvpool.tile([128, NPAIR * 128], F16, tag="kvf1")
                        nc.sync.dma_start(out=kvf1[:], in_=kv_out_d[1].ap())
                        kvf = kvpool.tile([128, NPAIR * 128], F16, tag="kvf")
                        nc.vector.tensor_tensor(kvf[:], kvf0[:], kvf1[:], ADD)
                        kvblk = kvpool.tile([128, NPAIR * 128], F16, tag="kvblk")
                        nc.vector.memset(kvblk[:], 0.0)
                        nc.vector.tensor_copy(
                            kvblk[0:64, :].rearrange(
                                "p (t f) -> p t f", t=NPAIR
                            )[:, :, 0:64],
                            kvf[0:64, :].rearrange("p (t f) -> p t f", t=NPAIR)[
                                :, :, 0:64
                            ],
                        )
                        nc.vector.tensor_copy(
                            kvblk[64:128, :].rearrange(
                                "p (t f) -> p t f", t=NPAIR
                            )[:, :, 64:128],
                            kvf[64:128, :].rearrange(
                                "p (t f) -> p t f", t=NPAIR
                            )[:, :, 64:128],
                        )
                        for half in range(2):
                            for hp in range(NPAIR):
                                fs = slice(hp * DIM + half * 512,
                                           hp * DIM + (half + 1) * 512)
                                g_ps = psO.tile([128, 512], F32, tag="o_ps")
                                nc.tensor.matmul(
                                    g_ps[:],
                                    kvblk[:, hp * 128 : (hp + 1) * 128],
                                    wo_all[:, fs],
                                    start=True,
                                    stop=True,
                                )
                                nc.scalar.activation(
                                    G_all[:, fs], g_ps[:], Copy
                                )

                    def _emit_out(ct_, qh_):
                        cs_ = slice(ct_ * 512, (ct_ + 1) * 512)
                        o_all = sbQ.tile([128, NJ * 512], F16, tag="o_all")
                        outv = out_d.rearrange("(t p) c -> p t c", p=128)
                        for et in range(NJ):
                            elo = et * 128
                            o_ps = psO.tile([128, 512], F32, tag="o_ps")
                            for jt in range(NJ):
                                nc.tensor.matmul(
                                    o_ps[:],
                                    G_all[
                                        :, jt * DIM + elo : jt * DIM + elo + 128
                                    ],
                                    qh_[:, jt * 512 : (jt + 1) * 512],
                                    start=(jt == 0),
                                    stop=(jt == NJ - 1),
                                )
                            nc.scalar.activation(
                                o_all[:, et * 512 : (et + 1) * 512], o_ps[:], Copy
                            )
                            # per-et DMA so the tail shrinks to one et
                            nc.sync.dma_start(
                                out=outv[:, et, cs_],
                                in_=o_all[:, et * 512 : (et + 1) * 512],
                            )

                    # emission order: pass1(0), pass2(0), pass1(1), pass2(1),
                    # G, out(0), out(1) — G sits behind the full q pipeline
                    # of independent PE work to hide the AllReduce latency.
                    q0, n0 = _emit_pass1(0, early=p1_early)
                    qh0 = _emit_pass2(0, q0, n0)
                    q1, n1 = _emit_pass1(1)
                    # G before pass2(1): its 16 Act evictions overlap
                    # pass2(1)'s PE/DVE work (Act is otherwise idle there)
                    _emit_G()
                    qh1 = _emit_pass2(1, q1, n1)
                    _emit_out(0, qh0)
                    _emit_out(1, qh1)

    nc.compile()
    return nc


_NC_CACHE = None


def _get_nc():
    global _NC_CACHE
    if _NC_CACHE is None:
        _NC_CACHE = build_nc()
    return _NC_CACHE


def _plan_rows(mask):
    """Split each batch's unmasked row indices over its 4 cores.
    Returns rows[core] = np.ndarray of original row ids (len <= R)."""
    mask = np.asarray(mask)
    rows = []
    for b in range(B):
        idx = np.flatnonzero(mask[b] != 0)
        n = len(idx)
        assert n <= 4 * R, f"unmasked rows {n} exceed capacity {4 * R}"
        szs = [n // 4 + (1 if i < n % 4 else 0) for i in range(4)]
        off = 0
        for cc in range(4):
            rows.append(idx[off : off + szs[cc]])
            off += szs[cc]
    return rows


def make_in_maps(x, mask, Wq, Wk, Wv, Wo, norm_const):
    x = np.asarray(x, np.float32)
    mask = np.asarray(mask)
    Wq = np.asarray(Wq, np.float32)
    Wk = np.asarray(Wk, np.float32)
    Wv = np.asarray(Wv, np.float32)
    Wo = np.asarray(Wo, np.float32)
    norm_const = np.asarray(norm_const, np.float32).reshape(H)

    sig = 1.0 / (1.0 + np.exp(-norm_const.astype(np.float64)))
    svec = np.float64(C) ** (-sig)  # [H]
    s_cols = np.repeat(svec, HD)  # [DIM]

    f16 = np.float16
    WkT = np.ascontiguousarray(Wk.T).astype(f16)
    WvT = np.ascontiguousarray((Wv * s_cols[:, None].astype(np.float32)).T).astype(
        f16
    )
    WqT = np.ascontiguousarray(Wq.T).astype(f16)
    WoT = np.ascontiguousarray(Wo.T).astype(f16)

    inv_freq = 1.0 / (
        ROPE_THETA ** (np.arange(0, HD, 2, dtype=np.float64) / HD)
    )  # [32]
    freq_of_j = np.repeat(inv_freq, 2)  # [64] interleaved

    ind16T = np.zeros((DIM, 16), f16)
    for jt in range(NJ):
        for kk in range(128):
            ind16T[jt * 128 + kk, 2 * jt + (kk >= 64)] = 1.0

    ind16 = np.zeros((16, DIM), f16)
    for jt in range(NJ):
        for m in range(128):
            ind16[2 * jt + (m >= 64), jt * 128 + m] = 1.0

    Pmat = np.zeros((128, 128), f16)
    for i in range(64):
        Pmat[2 * i + 1, 2 * i] = -1.0  # out[2i] = -q[2i+1]
        Pmat[2 * i, 2 * i + 1] = 1.0  # out[2i+1] = q[2i]

    rows_per_core = _plan_rows(mask)

    in_maps = []
    for core in range(N_CORES):
        b = core // (N_CORES // B)
        rows = rows_per_core[core]
        sz = len(rows)
        pos = np.zeros(R, np.float64)
        pos[:sz] = rows.astype(np.float64)

        # gathered x columns; pads filled with 1.0 (khat row is zeroed by
        # the mask so pads contribute nothing)
        xTc = np.ones((DIM, R), f16)
        xTc[:, :sz] = x[b, rows, :].T.astype(f16)

        angC = pos[:, None] * freq_of_j[None, :]  # [R, 64]
        cosCc = np.cos(angC).astype(f16)
        sinCc = np.sin(angC).astype(np.float32)
        # sign fold for the swap formulation: even j -> -sin, odd j -> +sin
        sinCc[:, 0::2] *= -1.0
        sinCc = sinCc.astype(f16)

        angF = freq_of_j[:, None] * pos[None, :]  # [64, R]
        angF2 = np.concatenate([angF, angF], axis=0)  # [128, R]
        cosFc = np.cos(angF2).astype(f16)
        sinFc = np.sin(angF2).astype(f16)

        mrow = np.zeros(R, np.float32)
        mrow[:sz] = 1.0
        maskCc = np.ascontiguousarray(mrow.reshape(NC_T, 128).T)  # [128, NC_T]

        in_maps.append(
            {
                "xT": xTc,
                "WkT": WkT,
                "WvT": WvT,
                "WqT": WqT,
                "WoT": WoT,
                "cosC": cosCc,
                "sinC": sinCc,
                "cosF": cosFc,
                "sinF": sinFc,
                "maskC": maskCc,
                "ind16T": ind16T,
                "ind16": ind16,
                "Pmat": Pmat,
            }
        )
    return in_maps


def assemble_output(results, mask):
    rows_per_core = _plan_rows(mask)
    out = np.zeros((B, C, DIM), np.float32)
    for core in range(N_CORES):
        b = core // (N_CORES // B)
        rows = rows_per_core[core]
        sz = len(rows)
        out[b, rows, :] = results[core]["out"].T[:sz].astype(np.float32)
    return out


def kernel(x, mask, Wq, Wk, Wv, Wo, norm_const):
    nc = _get_nc()
    in_maps = make_in_maps(x, mask, Wq, Wk, Wv, Wo, norm_const)
    res = run_bass_kernel_spmd(nc, in_maps, list(range(N_CORES)))
    return assemble_output(res.results, mask)
